# revision 8
# baseline (speedup 1.0000x reference)
"""Trainium2 Bass kernel for LightGCN-style GNN message passing (8 NeuronCores).

Contract: kernel(**inputs) takes FULL unsharded numpy inputs (as produced by the
problem's setup_inputs) and returns the FULL [100000] float32 output.

Design summary
--------------
Node "positions": the 15000 nodes (10000 users + 5000 movies) are permuted into
a padded 15360-slot position space with a [user,user,movie] interleave so that
any contiguous range holds users:movies 2:1.  Core k owns positions
[1920k, 1920(k+1)): 1875 real + 45 dummy.  Each core's range splits into 8
Q7-group ranges of 240 positions, each processed in 8 chunks of 30 rows.

Per propagation layer (x3):
  - gpsimd.ap_gather pulls E_T columns (f32, table [128, 15360] = 8 replicas of
    [16, 15360]) with an independent per-Q7-core index stream (edge cols).
  - ACT casts the bf16 edge values to f32, DVE multiplies and prefix-scans
    (tensor_tensor_scan) the message stream, a small boundary ap_gather +
    shifted subtract yields per-destination-row segment sums.
  - Stripes are AllGather'd across the 8 cores and the replicated table is
    rebuilt by DMA.

Encoders run as PE transposes + bf16 matmuls (W1 stationary per k-chunk,
features moving), producing E_T stripes directly.  Prediction gathers
user/item embedding columns (one ap_gather, group-pairs = batch slices) and
runs the tiny MLP via 32-row matmuls at partition offsets 32j.
"""

import os
import numpy as np
import ml_dtypes

import concourse.bass as bass
import concourse.bacc as bacc
import concourse.tile as tile
import concourse.mybir as mybir
from concourse.bass_utils import run_bass_kernel_spmd

F32 = mybir.dt.float32
BF16 = mybir.dt.bfloat16
I16 = mybir.dt.int16

# ---------------- problem constants (hardcoded per spec) ----------------
NU, NM = 10000, 5000          # users, movies
NN = NU + NM                  # 15000 nodes
NE = 1500000                  # edges
EMB, HID = 16, 64
BATCH = 100000
N_LAYERS = 3

NCORES = 8
NPOS = 15360                  # padded position space
CPP = 1920                    # positions per core
RPC = 1875                    # real positions per core
NG = 8                        # q7 groups per core
GPOS = 240                    # positions per group
R = 30                        # rows per chunk (per group)
NCHUNK = 8                    # chunks per layer (240/30)
RB = 32                       # boundary idx count per chunk (30 + 2 pad)

NU_PC, NM_PC = 1250, 625      # real users/movies per core
NU_PAD, NM_PAD = 1280, 640    # padded (/128 chunking)
KU_PAD, KM_PAD = 5120, 10112  # feature dims padded to /128 (40, 79 chunks)
NKC_U, NKC_M = 40, 79

BPC = BATCH // NCORES         # 12500 batch per core
NSL = 4                       # slices per core (group pairs)
SLB = BPC // NSL              # 3125
SLBP = 3136                   # padded to /16
STRIPE_W = 1936               # stripe0 width (CPP + movie-AP slack)


def _ru16(x):
    return (int(x) + 15) // 16 * 16


def _wrap16(vals, ncols):
    """idx list -> [16, ncols] wrapped layout (entry i at [i%16, i//16])."""
    out = np.zeros((16, ncols), dtype=vals.dtype)
    n = len(vals)
    full = np.zeros(ncols * 16, dtype=vals.dtype)
    full[:n] = vals
    out[:] = full.reshape(ncols, 16).T
    return out


def _host_prep(inputs):
    """All index/layout preprocessing. Returns (meta, in_maps)."""
    user_feats = np.asarray(inputs["user_feats"], np.float32)
    movie_feats = np.asarray(inputs["movie_feats"], np.float32)
    rows = np.asarray(inputs["graph_rows"], np.int64)
    cols = np.asarray(inputs["graph_cols"], np.int64)
    vals = np.asarray(inputs["graph_vals"], np.float32)
    users = np.asarray(inputs["users"], np.int64)
    items = np.asarray(inputs["items"], np.int64)

    # node id -> padded position
    u = np.arange(NU)
    pos_user = 1920 * (u // NU_PC) + 3 * ((u % NU_PC) // 2) + (u % NU_PC) % 2
    m = np.arange(NM)
    pos_movie = 1920 * (m // NM_PC) + 3 * (m % NM_PC) + 2
    pos_of_node = np.concatenate([pos_user, pos_movie + 0])  # movie node id = 10000+m
    pos_of_node[NU:] = pos_movie

    r_pos = pos_of_node[rows]
    c_pos = pos_of_node[cols].astype(np.int16)

    order = np.argsort(r_pos, kind="stable")
    c_sorted = c_pos[order]
    v_sorted = vals[order]

    deg = np.bincount(r_pos, minlength=NPOS)
    cum = np.zeros(NPOS + 1, np.int64)
    cum[1:] = np.cumsum(deg)

    # chunk widths W_c: global max over (core, group) for each chunk index c
    starts = np.arange(0, NPOS, R)
    slots_all = (cum[starts + R] - cum[starts]).reshape(NCORES, NG, NCHUNK)
    W = [_ru16(slots_all[:, :, c].max()) for c in range(NCHUNK)]
    TOT = int(sum(W))
    cumW = np.zeros(NCHUNK + 1, np.int64)
    cumW[1:] = np.cumsum(W)

    gidx_all, gval_all, bidx_all = [], [], []
    for s in range(NCORES):
        gidx = np.zeros((128, TOT // 16), np.int16)
        gval8 = np.zeros((NG, TOT), np.float32)
        bidx = np.zeros((128, NCHUNK * RB // 16), np.int16)
        for h in range(NG):
            for c in range(NCHUNK):
                p0 = 1920 * s + 240 * h + R * c
                e0, e1 = cum[p0], cum[p0 + R]
                nsl = int(e1 - e0)
                wc = W[c]
                base16 = int(cumW[c] // 16)
                gidx[16 * h:16 * h + 16, base16:base16 + wc // 16] = _wrap16(
                    c_sorted[e0:e1], wc // 16)
                gval8[h, cumW[c]:cumW[c] + nsl] = v_sorted[e0:e1]
                # boundary end positions (chunk-local, +16 for zero prefix)
                ends = (cum[p0 + 1:p0 + R + 1] - e0 - 1 + 16).astype(np.int16)
                ends_p = np.concatenate([ends, ends[-1:].repeat(RB - R)])
                bb = c * (RB // 16)
                bidx[16 * h:16 * h + 16, bb:bb + RB // 16] = _wrap16(ends_p, RB // 16)
        gval = np.repeat(gval8, 16, axis=0).astype(ml_dtypes.bfloat16)
        gidx_all.append(gidx)
        gval_all.append(gval)
        bidx_all.append(bidx)

    # prediction gather indices: [128, SLBP//16]; group 2j=users, 2j+1=items
    pu = pos_of_node[users].astype(np.int16)
    pi = pos_of_node[NU + items].astype(np.int16)
    uidx_all = []
    for s in range(NCORES):
        uidx = np.zeros((128, SLBP // 16), np.int16)
        for j in range(NSL):
            b0 = BPC * s + SLB * j
            uidx[32 * j:32 * j + 16, :] = _wrap16(pu[b0:b0 + SLB], SLBP // 16)
            uidx[32 * j + 16:32 * j + 32, :] = _wrap16(pi[b0:b0 + SLB], SLBP // 16)
        # slice 3 duplicated at partition base 0 (PE rejects base partition 96)
        uidx_b = np.zeros((128, SLBP // 16), np.int16)
        uidx_b[0:32, :] = uidx[96:128, :]
        uidx_all.append(np.concatenate([uidx, uidx_b], axis=1))

    # stds/means table at user positions
    sm = np.zeros((2, NPOS), np.float32)
    sm[0, pos_user] = np.asarray(inputs["norm_stds"], np.float32)
    sm[1, pos_user] = np.asarray(inputs["norm_means"], np.float32)

    # per-core padded feature shards
    uf_all, mf_all = [], []
    for s in range(NCORES):
        uf = np.zeros((NU_PAD, KU_PAD), np.float32)
        uf[:NU_PC, :NM] = user_feats[NU_PC * s:NU_PC * (s + 1)]
        mf = np.zeros((NM_PAD, KM_PAD), np.float32)
        mf[:NM_PC, :NU] = movie_feats[NM_PC * s:NM_PC * (s + 1)]
        uf_all.append(uf)
        mf_all.append(mf)

    bf = ml_dtypes.bfloat16
    uw1 = np.zeros((KU_PAD, HID), bf)
    uw1[:NM] = np.asarray(inputs["uw1"], np.float32).astype(bf)
    mw1 = np.zeros((KM_PAD, HID), bf)
    mw1[:NU] = np.asarray(inputs["mw1"], np.float32).astype(bf)
    uw2 = np.asarray(inputs["uw2"], np.float32).astype(bf)
    mw2 = np.asarray(inputs["mw2"], np.float32).astype(bf)
    ident = np.eye(128, dtype=bf)

    shared = {
        "uw1": uw1, "mw1": mw1, "uw2": uw2, "mw2": mw2,
        "ub1": np.asarray(inputs["ub1"], np.float32).reshape(HID, 1),
        "ub2": np.asarray(inputs["ub2"], np.float32).reshape(EMB, 1),
        "mb1": np.asarray(inputs["mb1"], np.float32).reshape(HID, 1),
        "mb2": np.asarray(inputs["mb2"], np.float32).reshape(EMB, 1),
        "fw1": np.tile(np.asarray(inputs["fw1"], np.float32), (4, 1)),  # [128,64]
        "fw2": np.asarray(inputs["fw2"], np.float32).reshape(HID, 1),
        "fb1": np.asarray(inputs["fb1"], np.float32).reshape(HID, 1),
        "fb2": np.asarray(inputs["fb2"], np.float32).reshape(1, 1),
        "sm": sm, "ident": ident,
    }
    in_maps = []
    for s in range(NCORES):
        im = dict(shared)
        im.update({
            "ufeats": uf_all[s], "mfeats": mf_all[s],
            "gidx": gidx_all[s], "gvals": gval_all[s], "bidx": bidx_all[s],
            "uidx": uidx_all[s],
        })
        in_maps.append(im)
    meta = {"W": W, "TOT": TOT, "cumW": cumW,
            "fb2": float(np.asarray(inputs["fb2"]).ravel()[0])}
    return meta, in_maps


def _build_program(meta):
    W, TOT, cumW = meta["W"], meta["TOT"], meta["cumW"]
    Wmax = max(W)

    nc = bacc.Bacc("TRN2", target_bir_lowering=False, debug=False,
                   num_devices=NCORES)

    d = {}
    def dram(name, shape, dt, kind="ExternalInput"):
        d[name] = nc.dram_tensor(name, list(shape), dt, kind=kind).ap()
        return d[name]

    dram("ufeats", (NU_PAD, KU_PAD), F32)
    dram("mfeats", (NM_PAD, KM_PAD), F32)
    dram("uw1", (KU_PAD, HID), BF16)
    dram("mw1", (KM_PAD, HID), BF16)
    dram("uw2", (HID, EMB), BF16)
    dram("mw2", (HID, EMB), BF16)
    for nm_, sh in [("ub1", (HID, 1)), ("ub2", (EMB, 1)), ("mb1", (HID, 1)),
                    ("mb2", (EMB, 1)), ("fb1", (HID, 1)), ("fb2", (1, 1)),
                    ("fw2", (HID, 1))]:
        dram(nm_, sh, F32)
    dram("fw1", (128, HID), F32)
    dram("sm", (2, NPOS), F32)
    dram("ident", (128, 128), BF16)
    dram("gidx", (128, TOT // 16), I16)
    dram("gvals", (128, TOT), BF16)
    dram("bidx", (128, NCHUNK * RB // 16), I16)
    dram("uidx", (128, 2 * (SLBP // 16)), I16)
    out_dram = dram("out", (NSL * SLBP,), F32, kind="ExternalOutput")

    RELU = mybir.ActivationFunctionType.Relu
    ADD = mybir.AluOpType.add
    SUB = mybir.AluOpType.subtract
    MULT = mybir.AluOpType.mult
    BYP = mybir.AluOpType.bypass

    with tile.TileContext(nc) as tc:
        with tc.tile_pool(name="const", bufs=1) as cp, \
             tc.tile_pool(name="dram", bufs=1, space="DRAM") as dp:
            # ---- resident tensors ----
            w2u = cp.tile([HID, EMB], BF16)
            nc.sync.dma_start(out=w2u[:], in_=d["uw2"][:])
            w2m = cp.tile([HID, EMB], BF16)
            nc.sync.dma_start(out=w2m[:], in_=d["mw2"][:])
            small = {}
            for nm_, sh in [("ub1", (HID, 1)), ("ub2", (EMB, 1)), ("mb1", (HID, 1)),
                            ("mb2", (EMB, 1)), ("fb1", (HID, 1)), ("fb2", (1, 1)),
                            ("fw2", (HID, 1)), ("fw1", (128, HID))]:
                t = cp.tile(list(sh), F32, tag=f"sm_{nm_}")
                nc.sync.dma_start(out=t[:], in_=d[nm_][:])
                small[nm_] = t
            gidx = cp.tile([128, TOT // 16], I16)
            nc.sync.dma_start(out=gidx[:], in_=d["gidx"][:])
            bidx = cp.tile([128, NCHUNK * RB // 16], I16)
            nc.sync.dma_start(out=bidx[:], in_=d["bidx"][:])
            uidx = cp.tile([128, 2 * (SLBP // 16)], I16)
            nc.sync.dma_start(out=uidx[:], in_=d["uidx"][:])

            table = cp.tile([128, NPOS], F32)        # replicated E_T gather table

            bounce_in = dp.tile([16, CPP], F32)
            bounce_out = dp.tile([128, CPP], F32)
            acc_dram = dp.tile([16, CPP], F32)       # running sum of E0..E3 (DRAM, CCE adds)

            # ================= encoders =================
            with tc.tile_pool(name="encw", bufs=1) as ewp, \
                 tc.tile_pool(name="feats", bufs=2) as fp, \
                 tc.tile_pool(name="ft", bufs=3) as ftp, \
                 tc.tile_pool(name="hsb", bufs=2) as hp, \
                 tc.tile_pool(name="tps", bufs=2, space="PSUM") as tpp, \
                 tc.tile_pool(name="hps", bufs=2, space="PSUM") as hpp, \
                 tc.tile_pool(name="eps", bufs=2, space="PSUM") as epp:
                ident = ewp.tile([128, 128], BF16)
                nc.sync.dma_start(out=ident[:], in_=d["ident"][:])
                w1u = ewp.tile([128, NKC_U, HID], BF16)
                nc.sync.dma_start(out=w1u[:], in_=d["uw1"].rearrange("(c p) h -> p c h", p=128))
                w1m = ewp.tile([128, NKC_M, HID], BF16)
                nc.sync.dma_start(out=w1m[:], in_=d["mw1"].rearrange("(c p) h -> p c h", p=128))
                stripe0 = ewp.tile([16, STRIPE_W], F32)  # encoder E0 stripe
                for which in range(2):
                    if which == 0:
                        feats, kpad, nkc, nrc = d["ufeats"], KU_PAD, NKC_U, NU_PAD // 128
                        w1sb, w2sb, b1, b2 = w1u, w2u, small["ub1"], small["ub2"]
                    else:
                        feats, kpad, nkc, nrc = d["mfeats"], KM_PAD, NKC_M, NM_PAD // 128
                        w1sb, w2sb, b1, b2 = w1m, w2m, small["mb1"], small["mb2"]
                    for rc in range(nrc):
                        fb = fp.tile([128, kpad], BF16, tag="fb")
                        # SWDGE cast-DMA f32 -> bf16
                        nc.gpsimd.dma_start(out=fb[:], in_=feats[rc * 128:(rc + 1) * 128, :])
                        hps = hpp.tile([HID, 128], F32)
                        for kc in range(nkc):
                            tps = tpp.tile([128, 128], BF16)
                            nc.tensor.transpose(tps[:], fb[:, kc * 128:(kc + 1) * 128], ident[:])
                            ftT = ftp.tile([128, 128], BF16)
                            nc.any.tensor_copy(ftT[:], tps[:])
                            nc.tensor.matmul(hps[:], w1sb[:, kc, :], ftT[:],
                                             start=(kc == 0), stop=(kc == nkc - 1))
                        h_sb = hp.tile([HID, 128], BF16)
                        nc.scalar.activation(h_sb[:], hps[:], RELU, bias=b1[:])
                        eps = epp.tile([EMB, 128], F32)
                        nc.tensor.matmul(eps[:], w2sb[:], h_sb[:])
                        # write into interleaved stripe positions
                        if which == 0:
                            a = 192 * rc
                            outap = stripe0[:, a:a + 192].rearrange(
                                "p (x y) -> p x y", y=3)[:, :, 0:2]
                            inap = eps[:].rearrange("p (x y) -> p x y", y=2)
                        else:
                            a = 384 * rc + 2
                            outap = stripe0[:, a:a + 384].rearrange(
                                "p (x y) -> p x y", y=3)[:, :, 0:1]
                            inap = eps[:].rearrange("p (x y) -> p x y", y=1)
                        nc.scalar.activation(outap, inap, RELU, bias=b2[:])

                # E0 exchange (inside encoder scope: stripe0 freed after)
                nc.sync.dma_start(out=bounce_in[:], in_=stripe0[:, :CPP])
                nc.sync.dma_start(out=acc_dram[:], in_=stripe0[:, :CPP])
                nc.gpsimd.collective_compute(
                    "AllGather", BYP, replica_groups=[list(range(NCORES))],
                    ins=[bounce_in.opt()], outs=[bounce_out.opt()])
                for g in range(NG):
                    for s in range(NCORES):
                        nc.sync.dma_start(
                            out=table[16 * g:16 * g + 16, CPP * s:CPP * (s + 1)],
                            in_=bounce_out[16 * s:16 * s + 16, :])

            # ================= propagation =================
            with tc.tile_pool(name="gbuf", bufs=2) as gp, \
                 tc.tile_pool(name="gv", bufs=2) as gvp, \
                 tc.tile_pool(name="cbuf", bufs=2) as cbp, \
                 tc.tile_pool(name="bnd", bufs=2) as bp, \
                 tc.tile_pool(name="stripes", bufs=2) as sp:
                for layer in range(N_LAYERS):
                    stripe = sp.tile([128, R * NCHUNK + 2], F32, tag="stripe")
                    for c in range(NCHUNK):
                        wc = W[c]
                        gb = gp.tile([128, Wmax], F32, tag="gb")
                        nc.gpsimd.ap_gather(
                            gb[:, :wc], table[:], gidx[:, cumW[c] // 16:cumW[c + 1] // 16],
                            channels=128, num_elems=NPOS, d=1, num_idxs=wc)
                        gv = gvp.tile([128, Wmax], BF16, tag="gv")
                        nc.sync.dma_start(out=gv[:, :wc],
                                          in_=d["gvals"][:, cumW[c]:cumW[c] + wc])
                        cb = cbp.tile([128, 16 + Wmax], F32, tag="cb")
                        nc.vector.memset(cb[:, 0:16], 0.0)
                        # cast vals bf16->f32 into cum buffer, multiply by gather
                        nc.scalar.activation(cb[:, 16:16 + wc], gv[:, :wc],
                                             mybir.ActivationFunctionType.Copy)
                        nc.vector.tensor_tensor(cb[:, 16:16 + wc], cb[:, 16:16 + wc],
                                                gb[:, :wc], MULT)
                        nc.vector.tensor_tensor_scan(cb[:, 16:16 + wc], cb[:, 16:16 + wc],
                                                     cb[:, 16:16 + wc], 0.0, ADD, BYP)
                        bnd = bp.tile([128, 1 + RB], F32, tag="bnd")
                        nc.vector.memset(bnd[:, 0:1], 0.0)
                        nc.gpsimd.ap_gather(
                            bnd[:, 1:1 + RB], cb[:, :16 + wc],
                            bidx[:, c * (RB // 16):(c + 1) * (RB // 16)],
                            channels=128, num_elems=16 + wc, d=1, num_idxs=RB)
                        nc.vector.tensor_tensor(stripe[:, R * c:R * c + RB],
                                                bnd[:, 1:1 + RB], bnd[:, 0:RB], SUB)
                    # stripe -> bounce (group layout -> simple layout)
                    for h in range(NG):
                        nc.sync.dma_start(out=bounce_in[:, GPOS * h:GPOS * (h + 1)],
                                          in_=stripe[16 * h:16 * h + 16, 0:GPOS])
                    nc.gpsimd.dma_start(out=acc_dram[:], in_=bounce_in[:],
                                        accum_op=mybir.AluOpType.add)
                    if layer < N_LAYERS - 1:
                        nc.gpsimd.collective_compute(
                            "AllGather", BYP, replica_groups=[list(range(NCORES))],
                            ins=[bounce_in.opt()], outs=[bounce_out.opt()])
                        for g in range(NG):
                            for s in range(NCORES):
                                nc.sync.dma_start(
                                    out=table[16 * g:16 * g + 16, CPP * s:CPP * (s + 1)],
                                    in_=bounce_out[16 * s:16 * s + 16, :])

            # light = acc / 4 -> exchange -> light table
            with tc.tile_pool(name="lacc", bufs=1) as lap:
                accl = lap.tile([16, CPP], F32)
                nc.sync.dma_start(out=accl[:], in_=acc_dram[:])
                nc.vector.tensor_scalar_mul(accl[:], accl[:], 0.25)
                nc.sync.dma_start(out=bounce_in[:], in_=accl[:])
                nc.gpsimd.collective_compute(
                    "AllGather", BYP, replica_groups=[list(range(NCORES))],
                    ins=[bounce_in.opt()], outs=[bounce_out.opt()])
                for g in range(NG):
                    for s in range(NCORES):
                        nc.sync.dma_start(
                            out=table[16 * g:16 * g + 16, CPP * s:CPP * (s + 1)],
                            in_=bounce_out[16 * s:16 * s + 16, :])

            # ================= prediction =================
            with tc.tile_pool(name="pg", bufs=1) as pgp, \
                 tc.tile_pool(name="posb", bufs=1) as osp, \
                 tc.tile_pool(name="ph", bufs=2) as php, \
                 tc.tile_pool(name="pps", bufs=2, space="PSUM") as ppp, \
                 tc.tile_pool(name="gps", bufs=1, space="PSUM") as gpp:
                ui_a, ui_b = uidx[:, :SLBP // 16], uidx[:, SLBP // 16:]
                pg = pgp.tile([128, SLBP], F32, tag="pg")
                nc.gpsimd.ap_gather(pg[:], table[:], ui_a,
                                    channels=128, num_elems=NPOS, d=1, num_idxs=SLBP)
                pgb = pgp.tile([128, SLBP], F32, tag="pgb")
                nc.gpsimd.ap_gather(pgb[:], table[:], ui_b,
                                    channels=128, num_elems=NPOS, d=1, num_idxs=SLBP)
                # stds round: overwrite table row 16g with stds, gather
                for g in range(NG):
                    nc.sync.dma_start(out=table[16 * g:16 * g + 1, :],
                                      in_=d["sm"][0:1, :])
                pg2 = pgp.tile([128, SLBP], F32, tag="pg2")
                nc.gpsimd.ap_gather(pg2[:], table[:], ui_a,
                                    channels=128, num_elems=NPOS, d=1, num_idxs=SLBP)
                pg2b = pgp.tile([128, SLBP], F32, tag="pg2b")
                nc.gpsimd.ap_gather(pg2b[:], table[:], ui_b,
                                    channels=128, num_elems=NPOS, d=1, num_idxs=SLBP)
                # means round
                for g in range(NG):
                    nc.sync.dma_start(out=table[16 * g:16 * g + 1, :],
                                      in_=d["sm"][1:2, :])
                pg3 = pgp.tile([128, SLBP], F32, tag="pg3")
                nc.gpsimd.ap_gather(pg3[:], table[:], ui_a,
                                    channels=128, num_elems=NPOS, d=1, num_idxs=SLBP)
                pg3b = pgp.tile([128, SLBP], F32, tag="pg3b")
                nc.gpsimd.ap_gather(pg3b[:], table[:], ui_b,
                                    channels=128, num_elems=NPOS, d=1, num_idxs=SLBP)

                osbA = osp.tile([128, SLBP], F32, tag="osbA")
                osbB = osp.tile([1, SLBP], F32, tag="osbB")
                NCH = [512] * 6 + [64]
                for j in range(NSL):
                    if j < 3:
                        base, s_pg, s_std, s_mean = 32 * j, pg, pg2, pg3
                        orow = osbA[base:base + 1, :]
                    else:
                        base, s_pg, s_std, s_mean = 0, pgb, pg2b, pg3b
                        orow = osbB[0:1, :]
                    n0 = 0
                    for nn in NCH:
                        hps2 = ppp.tile([HID, 512], F32, tag="hps2")
                        nc.tensor.matmul(hps2[:, :nn], small["fw1"][base:base + 32, :],
                                         s_pg[base:base + 32, n0:n0 + nn])
                        h2 = php.tile([HID, 512], F32, tag="h2")
                        nc.scalar.activation(h2[:, :nn], hps2[:, :nn], RELU,
                                             bias=small["fb1"][:])
                        gps = gpp.tile([128, 512], F32, tag="gps")
                        nc.tensor.matmul(gps[base:base + 1, :nn], small["fw2"][:],
                                         h2[:, :nn])
                        gsl = orow[:, n0:n0 + nn]
                        nc.scalar.activation(gsl, gps[base:base + 1, :nn], RELU,
                                             bias=float(meta["fb2"]))
                        nc.vector.tensor_tensor(gsl, gsl,
                                                s_std[base:base + 1, n0:n0 + nn], MULT)
                        nc.any.tensor_tensor(gsl, gsl,
                                             s_mean[base:base + 1, n0:n0 + nn], ADD)
                        n0 += nn
                    nc.sync.dma_start(out=out_dram[SLBP * j:SLBP * (j + 1)], in_=orow)

    nc.compile()
    return nc


def kernel(**inputs) -> np.ndarray:
    meta, in_maps = _host_prep(inputs)
    nc = _build_program(meta)

    if os.environ.get("BASS_SIM"):
        import concourse.bass_interp as bass_interp
        sim = bass_interp.MultiCoreSim(nc, NCORES)
        for i in range(NCORES):
            for k, v in in_maps[i].items():
                sim.cores[i].tensor(k)[:] = v
        sim.simulate(check_with_hw=False)
        results = [{"out": np.array(sim.cores[i].tensor("out"))} for i in range(NCORES)]
    else:
        res = run_bass_kernel_spmd(nc, in_maps, list(range(NCORES)))
        results = res.results

    outs = []
    for s in range(NCORES):
        o = np.asarray(results[s]["out"], np.float32).reshape(NSL, SLBP)
        outs.append(o[:, :SLB].ravel())
    return np.concatenate(outs)


# revision 20
# speedup vs baseline: 18.7478x; 18.7478x over previous
"""Trainium2 Bass kernel for LightGCN-style GNN message passing (8 NeuronCores).

Contract: kernel(**inputs) takes FULL unsharded numpy inputs (as produced by the
problem's setup_inputs) and returns the FULL [100000] float32 output.

Design summary
--------------
Node "positions": the 15000 nodes (10000 users + 5000 movies) are permuted into
a padded 15360-slot position space with a [user,user,movie] interleave so that
any contiguous range holds users:movies 2:1.  Core k owns positions
[1920k, 1920(k+1)): 1875 real + 45 dummy.  Each core's range splits into 8
Q7-group ranges of 240 positions, each processed in 8 chunks of 30 rows.

Per propagation layer (x3):
  - gpsimd.ap_gather pulls E_T columns (f32, table [128, 15360] = 8 replicas of
    [16, 15360]) with an independent per-Q7-core index stream (edge cols).
  - ACT casts the bf16 edge values to f32, DVE multiplies and prefix-scans
    (tensor_tensor_scan) the message stream, a small boundary ap_gather +
    shifted subtract yields per-destination-row segment sums.
  - Stripes are AllGather'd across the 8 cores and the replicated table is
    rebuilt by DMA.

Encoders run as PE transposes + bf16 matmuls (W1 stationary per k-chunk,
features moving), producing E_T stripes directly.  Prediction gathers
user/item embedding columns (one ap_gather, group-pairs = batch slices) and
runs the tiny MLP via 32-row matmuls at partition offsets 32j.
"""

import os
import numpy as np
import ml_dtypes

import concourse.bass as bass
import concourse.bacc as bacc
import concourse.tile as tile
import concourse.mybir as mybir
from concourse.bass_utils import run_bass_kernel_spmd

F32 = mybir.dt.float32
BF16 = mybir.dt.bfloat16
I16 = mybir.dt.int16

# ---------------- problem constants (hardcoded per spec) ----------------
NU, NM = 10000, 5000          # users, movies
NN = NU + NM                  # 15000 nodes
NE = 1500000                  # edges
EMB, HID = 16, 64
BATCH = 100000
N_LAYERS = 3

NCORES = 8
NPOS = 15360                  # padded position space
CPP = 1920                    # positions per core
RPC = 1875                    # real positions per core
NG = 8                        # q7 groups per core
GPOS = 240                    # positions per group
R = 30                        # rows per chunk (per group)
NCHUNK = 8                    # chunks per layer (240/30)
RB = 32                       # boundary idx count per chunk (30 + 2 pad)

NU_PC, NM_PC = 1250, 625      # real users/movies per core
NU_PAD, NM_PAD = 1280, 640    # padded (/128 chunking)
KU_PAD, KM_PAD = 5120, 10112  # feature dims padded to /128 (40, 79 chunks)
NKC_U, NKC_M = 40, 79

BPC = BATCH // NCORES         # 12500 batch per core
NSL = 4                       # slices per core (group pairs)
SLB = BPC // NSL              # 3125
SLBP = 3136                   # padded to /16
STRIPE_W = 1936               # stripe0 width (CPP + movie-AP slack)


def _ru16(x):
    # 32 keeps every chunk's int16 idx-column base 4-byte aligned in SBUF
    # (the Q7 gather ucode reads idx pairs as 32-bit words).
    return (int(x) + 31) // 32 * 32


def _wrap16(vals, ncols):
    """idx list -> [16, ncols] wrapped layout (entry i at [i%16, i//16])."""
    out = np.zeros((16, ncols), dtype=vals.dtype)
    n = len(vals)
    full = np.zeros(ncols * 16, dtype=vals.dtype)
    full[:n] = vals
    out[:] = full.reshape(ncols, 16).T
    return out


def _host_prep(inputs):
    """All index/layout preprocessing. Returns (meta, in_maps)."""
    user_feats = np.asarray(inputs["user_feats"], np.float32)
    movie_feats = np.asarray(inputs["movie_feats"], np.float32)
    rows = np.asarray(inputs["graph_rows"], np.int64)
    cols = np.asarray(inputs["graph_cols"], np.int64)
    vals = np.asarray(inputs["graph_vals"], np.float32)
    users = np.asarray(inputs["users"], np.int64)
    items = np.asarray(inputs["items"], np.int64)

    # node id -> padded position
    u = np.arange(NU)
    pos_user = 1920 * (u // NU_PC) + 3 * ((u % NU_PC) // 2) + (u % NU_PC) % 2
    m = np.arange(NM)
    pos_movie = 1920 * (m // NM_PC) + 3 * (m % NM_PC) + 2
    pos_of_node = np.concatenate([pos_user, pos_movie + 0])  # movie node id = 10000+m
    pos_of_node[NU:] = pos_movie

    r_pos = pos_of_node[rows]
    c_pos = pos_of_node[cols].astype(np.int16)

    order = np.argsort(r_pos, kind="stable")
    c_sorted = c_pos[order]
    v_sorted = vals[order]

    deg = np.bincount(r_pos, minlength=NPOS)
    cum = np.zeros(NPOS + 1, np.int64)
    cum[1:] = np.cumsum(deg)

    # chunk widths W_c: global max over (core, group) for each chunk index c
    starts = np.arange(0, NPOS, R)
    slots_all = (cum[starts + R] - cum[starts]).reshape(NCORES, NG, NCHUNK)
    W = [_ru16(slots_all[:, :, c].max()) for c in range(NCHUNK)]
    TOT = int(sum(W))
    cumW = np.zeros(NCHUNK + 1, np.int64)
    cumW[1:] = np.cumsum(W)

    gidx_all, gval_all, bidx_all = [], [], []
    for s in range(NCORES):
        gidx = np.zeros((128, TOT // 16), np.int16)
        gval8 = np.zeros((NG, TOT), np.float32)
        bidx = np.zeros((128, NCHUNK * RB // 16), np.int16)
        for h in range(NG):
            for c in range(NCHUNK):
                p0 = 1920 * s + 240 * h + R * c
                e0, e1 = cum[p0], cum[p0 + R]
                nsl = int(e1 - e0)
                wc = W[c]
                base16 = int(cumW[c] // 16)
                gidx[16 * h:16 * h + 16, base16:base16 + wc // 16] = _wrap16(
                    c_sorted[e0:e1], wc // 16)
                gval8[h, cumW[c]:cumW[c] + nsl] = v_sorted[e0:e1]
                # boundary end positions (chunk-local, +16 for zero prefix)
                ends = (cum[p0 + 1:p0 + R + 1] - e0 - 1 + 16).astype(np.int16)
                ends_p = np.concatenate([ends, ends[-1:].repeat(RB - R)])
                bb = c * (RB // 16)
                bidx[16 * h:16 * h + 16, bb:bb + RB // 16] = _wrap16(ends_p, RB // 16)
        gval = np.repeat(gval8, 16, axis=0).astype(ml_dtypes.bfloat16)
        gidx_all.append(gidx)
        gval_all.append(gval)
        bidx_all.append(bidx)

    # prediction gather indices: [128, SLBP//16]; group 2j=users, 2j+1=items
    pu = pos_of_node[users].astype(np.int16)
    pi = pos_of_node[NU + items].astype(np.int16)
    uidx_all = []
    for s in range(NCORES):
        uidx = np.zeros((128, SLBP // 16), np.int16)
        for j in range(NSL):
            b0 = BPC * s + SLB * j
            uidx[32 * j:32 * j + 16, :] = _wrap16(pu[b0:b0 + SLB], SLBP // 16)
            uidx[32 * j + 16:32 * j + 32, :] = _wrap16(pi[b0:b0 + SLB], SLBP // 16)
        # slice 3 duplicated at partition base 0 (PE rejects base partition 96)
        uidx_b = np.zeros((128, SLBP // 16), np.int16)
        uidx_b[0:32, :] = uidx[96:128, :]
        uidx_all.append(np.concatenate([uidx, uidx_b], axis=1))

    # stds/means table at user positions
    sm = np.zeros((2, NPOS), np.float32)
    sm[0, pos_user] = np.asarray(inputs["norm_stds"], np.float32)
    sm[1, pos_user] = np.asarray(inputs["norm_means"], np.float32)

    # per-core padded feature shards
    uf_all, mf_all = [], []
    for s in range(NCORES):
        uf = np.zeros((NU_PAD, KU_PAD), np.float32)
        uf[:NU_PC, :NM] = user_feats[NU_PC * s:NU_PC * (s + 1)]
        mf = np.zeros((NM_PAD, KM_PAD), np.float32)
        mf[:NM_PC, :NU] = movie_feats[NM_PC * s:NM_PC * (s + 1)]
        uf_all.append(uf)
        mf_all.append(mf)

    bf = ml_dtypes.bfloat16
    uw1 = np.zeros((KU_PAD, HID), np.float32)
    uw1[:NM] = np.asarray(inputs["uw1"], np.float32)
    mw1 = np.zeros((KM_PAD, HID), np.float32)
    mw1[:NU] = np.asarray(inputs["mw1"], np.float32)
    uw2 = np.asarray(inputs["uw2"], np.float32).astype(bf)
    mw2 = np.asarray(inputs["mw2"], np.float32).astype(bf)
    ident = np.eye(128, dtype=bf)

    shared = {
        "uw1": uw1, "mw1": mw1, "uw2": uw2, "mw2": mw2,
        "ub1": np.asarray(inputs["ub1"], np.float32).reshape(HID, 1),
        "ub2": np.asarray(inputs["ub2"], np.float32).reshape(EMB, 1),
        "mb1": np.asarray(inputs["mb1"], np.float32).reshape(HID, 1),
        "mb2": np.asarray(inputs["mb2"], np.float32).reshape(EMB, 1),
        "fw1": np.tile(np.asarray(inputs["fw1"], np.float32), (4, 1)),  # [128,64]
        "fw2": np.asarray(inputs["fw2"], np.float32).reshape(HID, 1),
        "fb1": np.asarray(inputs["fb1"], np.float32).reshape(HID, 1),
        "fb2": np.asarray(inputs["fb2"], np.float32).reshape(1, 1),
        "sm": sm, "ident": ident,
    }
    in_maps = []
    for s in range(NCORES):
        im = dict(shared)
        im.update({
            "ufeats": uf_all[s], "mfeats": mf_all[s],
            "gidx": gidx_all[s], "gvals": gval_all[s], "bidx": bidx_all[s],
            "uidx": uidx_all[s],
        })
        in_maps.append(im)
    meta = {"W": W, "TOT": TOT, "cumW": cumW,
            "fb2": float(np.asarray(inputs["fb2"]).ravel()[0])}
    return meta, in_maps


def _build_program(meta):
    W, TOT, cumW = meta["W"], meta["TOT"], meta["cumW"]
    Wmax = max(W)
    SKIP_ENC = bool(os.environ.get("K_SKIP_ENC"))
    SKIP_PROP = bool(os.environ.get("K_SKIP_PROP"))
    SKIP_PRED = bool(os.environ.get("K_SKIP_PRED"))
    REP_PROP = int(os.environ.get("K_REP_PROP", "1"))
    REP_ENC = int(os.environ.get("K_REP_ENC", "1"))
    REP_PRED = int(os.environ.get("K_REP_PRED", "1"))
    NO_GATHER = bool(os.environ.get("K_NO_GATHER"))
    NO_DVE = bool(os.environ.get("K_NO_DVE"))
    NO_EXCH = bool(os.environ.get("K_NO_EXCH"))
    DEBUG = bool(os.environ.get("K_DEBUG"))

    nc = bacc.Bacc("TRN2", target_bir_lowering=False, debug=False,
                   num_devices=NCORES)

    d = {}
    def dram(name, shape, dt, kind="ExternalInput"):
        d[name] = nc.dram_tensor(name, list(shape), dt, kind=kind).ap()
        return d[name]

    dram("ufeats", (NU_PAD, KU_PAD), F32)
    dram("mfeats", (NM_PAD, KM_PAD), F32)
    dram("uw1", (KU_PAD, HID), F32)
    dram("mw1", (KM_PAD, HID), F32)
    dram("uw2", (HID, EMB), BF16)
    dram("mw2", (HID, EMB), BF16)
    for nm_, sh in [("ub1", (HID, 1)), ("ub2", (EMB, 1)), ("mb1", (HID, 1)),
                    ("mb2", (EMB, 1)), ("fb1", (HID, 1)), ("fb2", (1, 1)),
                    ("fw2", (HID, 1))]:
        dram(nm_, sh, F32)
    dram("fw1", (128, HID), F32)
    dram("sm", (2, NPOS), F32)
    dram("ident", (128, 128), BF16)
    dram("gidx", (128, TOT // 16), I16)
    dram("gvals", (128, TOT), BF16)
    dram("bidx", (128, NCHUNK * RB // 16), I16)
    dram("uidx", (128, 2 * (SLBP // 16)), I16)
    out_dram = dram("out", (NSL * SLBP,), F32, kind="ExternalOutput")
    if DEBUG:
        dram("dbg_s0", (16, CPP), F32, kind="ExternalOutput")
        dram("dbg_t1", (16, NPOS), F32, kind="ExternalOutput")
        dram("dbg_t2", (16, NPOS), F32, kind="ExternalOutput")
        dram("dbg_acc", (16, CPP), F32, kind="ExternalOutput")
        dram("dbg_st1", (128, R * NCHUNK + 2), F32, kind="ExternalOutput")
        dram("dbg_gb", (128, W[2]), F32, kind="ExternalOutput")
        dram("dbg_gv", (128, W[2]), F32, kind="ExternalOutput")
        dram("dbg_cb", (128, 16 + W[2]), F32, kind="ExternalOutput")
        dram("dbg_bnd", (128, 33), F32, kind="ExternalOutput")
        dram("dbg_t0", (16, NPOS), F32, kind="ExternalOutput")
        dram("dbg_tl", (16, NPOS), F32, kind="ExternalOutput")

    RELU = mybir.ActivationFunctionType.Relu
    ADD = mybir.AluOpType.add
    SUB = mybir.AluOpType.subtract
    MULT = mybir.AluOpType.mult
    BYP = mybir.AluOpType.bypass

    with tile.TileContext(nc) as tc:
        with tc.tile_pool(name="const", bufs=1) as cp, \
             tc.tile_pool(name="dram", bufs=1, space="DRAM") as dp:
            # ---- resident tensors ----
            w2u = cp.tile([HID, EMB], BF16)
            nc.sync.dma_start(out=w2u[:], in_=d["uw2"][:])
            w2m = cp.tile([HID, EMB], BF16)
            nc.sync.dma_start(out=w2m[:], in_=d["mw2"][:])
            small = {}
            for nm_, sh in [("ub1", (HID, 1)), ("ub2", (EMB, 1)), ("mb1", (HID, 1)),
                            ("mb2", (EMB, 1)), ("fb1", (HID, 1)), ("fb2", (1, 1)),
                            ("fw2", (HID, 1)), ("fw1", (128, HID))]:
                t = cp.tile(list(sh), F32, tag=f"sm_{nm_}")
                nc.sync.dma_start(out=t[:], in_=d[nm_][:])
                small[nm_] = t
            gidx = cp.tile([128, TOT // 16], I16)
            nc.sync.dma_start(out=gidx[:], in_=d["gidx"][:])
            bidx = cp.tile([128, NCHUNK * RB // 16], I16)
            nc.sync.dma_start(out=bidx[:], in_=d["bidx"][:])
            uidx = cp.tile([128, 2 * (SLBP // 16)], I16)
            nc.sync.dma_start(out=uidx[:], in_=d["uidx"][:])

            table = cp.tile([128, NPOS], F32)        # replicated E_T gather table

            bounce_in = dp.tile([16, CPP], F32)
            bounce_out = dp.tile([128, CPP], F32)
            acc_dram = dp.tile([16, CPP], F32)       # running sum of E0..E3 (DRAM, CCE adds)

            # ================= encoders =================
            with tc.tile_pool(name="encw", bufs=1) as ewp, \
                 tc.tile_pool(name="feats", bufs=2) as fp, \
                 tc.tile_pool(name="ft", bufs=3) as ftp, \
                 tc.tile_pool(name="hsb", bufs=2) as hp, \
                 tc.tile_pool(name="tps", bufs=2, space="PSUM") as tpp, \
                 tc.tile_pool(name="hps", bufs=2, space="PSUM") as hpp, \
                 tc.tile_pool(name="eps", bufs=2, space="PSUM") as epp:
                if SKIP_ENC:
                    nrc_scale = 0
                else:
                    nrc_scale = 1
                ident = ewp.tile([128, 128], BF16)
                nc.sync.dma_start(out=ident[:], in_=d["ident"][:])
                w1u = ewp.tile([128, NKC_U, HID], F32)
                nc.sync.dma_start(out=w1u[:], in_=d["uw1"].rearrange("(c p) h -> p c h", p=128))
                w1m = ewp.tile([128, NKC_M, HID], F32)
                nc.sync.dma_start(out=w1m[:], in_=d["mw1"].rearrange("(c p) h -> p c h", p=128))
                stripe0 = ewp.tile([16, STRIPE_W], F32)  # encoder E0 stripe
                if SKIP_ENC:
                    nc.vector.memset(stripe0[:], 0.0)
                for which in list(range(2)) * REP_ENC:
                    if which == 0:
                        feats, kpad, nkc, nrc = d["ufeats"], KU_PAD, NKC_U, nrc_scale * (NU_PAD // 128)
                        w1sb, w2sb, b1, b2 = w1u, w2u, small["ub1"], small["ub2"]
                    else:
                        feats, kpad, nkc, nrc = d["mfeats"], KM_PAD, NKC_M, nrc_scale * (NM_PAD // 128)
                        w1sb, w2sb, b1, b2 = w1m, w2m, small["mb1"], small["mb2"]
                    for rc in range(nrc):
                        fb = fp.tile([128, kpad], BF16, tag="fb")
                        # SWDGE cast-DMA f32 -> bf16
                        nc.gpsimd.dma_start(out=fb[:], in_=feats[rc * 128:(rc + 1) * 128, :])
                        hps = hpp.tile([HID, 128], F32)
                        for kc in range(nkc):
                            tps = tpp.tile([128, 128], BF16)
                            nc.tensor.transpose(tps[:], fb[:, kc * 128:(kc + 1) * 128], ident[:])
                            ftT = ftp.tile([128, 128], F32)
                            nc.vector.tensor_copy(ftT[:], tps[:])
                            nc.tensor.matmul(hps[:], w1sb[:, kc, :], ftT[:],
                                             start=(kc == 0), stop=(kc == nkc - 1))
                        h_sb = hp.tile([HID, 128], BF16)
                        nc.scalar.activation(h_sb[:], hps[:], RELU, bias=b1[:])
                        eps = epp.tile([EMB, 128], F32)
                        nc.tensor.matmul(eps[:], w2sb[:], h_sb[:])
                        # write into interleaved stripe positions
                        if which == 0:
                            a = 192 * rc
                            outap = stripe0[:, a:a + 192].rearrange(
                                "p (x y) -> p x y", y=3)[:, :, 0:2]
                            inap = eps[:].rearrange("p (x y) -> p x y", y=2)
                        else:
                            a = 384 * rc + 2
                            outap = stripe0[:, a:a + 384].rearrange(
                                "p (x y) -> p x y", y=3)[:, :, 0:1]
                            inap = eps[:].rearrange("p (x y) -> p x y", y=1)
                        nc.scalar.activation(outap, inap, RELU, bias=b2[:])

                # E0 exchange (inside encoder scope: stripe0 freed after)
                if DEBUG:
                    nc.sync.dma_start(out=d["dbg_s0"][:], in_=stripe0[:, :CPP])
                nc.sync.dma_start(out=bounce_in[:], in_=stripe0[:, :CPP])
                nc.sync.dma_start(out=acc_dram[:], in_=stripe0[:, :CPP])
                nc.gpsimd.collective_compute(
                    "AllGather", BYP, replica_groups=[list(range(NCORES))],
                    ins=[bounce_in.opt()], outs=[bounce_out.opt()])
                for g in range(NG):
                    for s in range(NCORES):
                        nc.sync.dma_start(
                            out=table[16 * g:16 * g + 16, CPP * s:CPP * (s + 1)],
                            in_=bounce_out[16 * s:16 * s + 16, :])

            if DEBUG:
                nc.sync.dma_start(out=d["dbg_t0"][:], in_=table[0:16, :])
            # ================= propagation =================
            with tc.tile_pool(name="gbuf", bufs=3) as gp, \
                 tc.tile_pool(name="gv", bufs=3) as gvp, \
                 tc.tile_pool(name="cbuf", bufs=3) as cbp, \
                 tc.tile_pool(name="bnd", bufs=3) as bp, \
                 tc.tile_pool(name="stripes", bufs=2) as sp:
                for layer in ([] if SKIP_PROP else list(range(N_LAYERS)) * REP_PROP):
                    stripe = sp.tile([128, R * NCHUNK + 2], F32, tag="stripe")
                    for c in range(NCHUNK):
                        wc = W[c]
                        gb = gp.tile([128, Wmax], F32, tag="gb")
                        if NO_GATHER:
                            nc.vector.memset(gb[:, :wc], 0.01)
                        else:
                            nc.gpsimd.ap_gather(
                                gb[:, :wc], table[:], gidx[:, cumW[c] // 16:cumW[c + 1] // 16],
                                channels=128, num_elems=NPOS, d=1, num_idxs=wc)
                        gv = gvp.tile([128, Wmax], BF16, tag="gv")
                        nc.sync.dma_start(out=gv[:, :wc],
                                          in_=d["gvals"][:, cumW[c]:cumW[c] + wc])
                        if DEBUG and layer == 0 and c == 2:
                            nc.gpsimd.dma_start(out=d["dbg_gv"][:], in_=gv[:, :wc])
                        cb = cbp.tile([128, 16 + Wmax], F32, tag="cb")
                        if NO_DVE:
                            nc.vector.memset(cb[:, 0:RB], 0.0)
                        nc.vector.memset(cb[:, 0:16], 0.0)
                        # cast vals bf16->f32 into cum buffer, multiply by gather
                        if not NO_DVE:
                            nc.scalar.activation(cb[:, 16:16 + wc], gv[:, :wc],
                                                 mybir.ActivationFunctionType.Copy)
                            nc.vector.tensor_tensor(cb[:, 16:16 + wc], cb[:, 16:16 + wc],
                                                    gb[:, :wc], MULT)
                            nc.vector.tensor_tensor_scan(cb[:, 16:16 + wc], cb[:, 16:16 + wc],
                                                         cb[:, 16:16 + wc], 0.0, ADD, BYP)
                        if DEBUG and layer == 0 and c == 2:
                            nc.sync.dma_start(out=d["dbg_gb"][:], in_=gb[:, :wc])
                            nc.sync.dma_start(out=d["dbg_cb"][:], in_=cb[:, :16 + wc])
                        bnd = bp.tile([128, 1 + RB], F32, tag="bnd")
                        nc.vector.memset(bnd[:, 0:1], 0.0)
                        if NO_DVE:
                            nc.vector.memset(bnd[:, 1:1 + RB], 0.0)
                        else:
                            nc.gpsimd.ap_gather(
                                bnd[:, 1:1 + RB], cb[:, :16 + wc],
                                bidx[:, c * (RB // 16):(c + 1) * (RB // 16)],
                                channels=128, num_elems=16 + wc, d=1, num_idxs=RB)
                        if DEBUG and layer == 0 and c == 2:
                            nc.sync.dma_start(out=d["dbg_bnd"][:], in_=bnd[:])
                        nc.vector.tensor_tensor(stripe[:, R * c:R * c + RB],
                                                bnd[:, 1:1 + RB], bnd[:, 0:RB], SUB)
                    # stripe -> bounce (group layout -> simple layout)
                    for h in range(NG):
                        nc.sync.dma_start(out=bounce_in[:, GPOS * h:GPOS * (h + 1)],
                                          in_=stripe[16 * h:16 * h + 16, 0:GPOS])
                    nc.gpsimd.dma_start(out=acc_dram[:], in_=bounce_in[:],
                                        accum_op=mybir.AluOpType.add)
                    if DEBUG and layer == 0:
                        nc.sync.dma_start(out=d["dbg_st1"][:], in_=stripe[:])
                    if layer < N_LAYERS - 1 and not NO_EXCH:
                        nc.gpsimd.collective_compute(
                            "AllGather", BYP, replica_groups=[list(range(NCORES))],
                            ins=[bounce_in.opt()], outs=[bounce_out.opt()])
                        for g in range(NG):
                            for s in range(NCORES):
                                nc.sync.dma_start(
                                    out=table[16 * g:16 * g + 16, CPP * s:CPP * (s + 1)],
                                    in_=bounce_out[16 * s:16 * s + 16, :])
                        if DEBUG and layer == 0:
                            nc.sync.dma_start(out=d["dbg_t1"][:], in_=table[0:16, :])
                        if DEBUG and layer == 1:
                            nc.sync.dma_start(out=d["dbg_t2"][:], in_=table[0:16, :])

            if DEBUG:
                nc.sync.dma_start(out=d["dbg_acc"][:], in_=acc_dram[:])
            # light = acc / 4 -> exchange -> light table
            with tc.tile_pool(name="lacc", bufs=1) as lap:
                accl = lap.tile([16, CPP], F32)
                nc.sync.dma_start(out=accl[:], in_=acc_dram[:])
                nc.vector.tensor_scalar_mul(accl[:], accl[:], 0.25)
                nc.sync.dma_start(out=bounce_in[:], in_=accl[:])
                nc.gpsimd.collective_compute(
                    "AllGather", BYP, replica_groups=[list(range(NCORES))],
                    ins=[bounce_in.opt()], outs=[bounce_out.opt()])
                for g in range(NG):
                    for s in range(NCORES):
                        nc.sync.dma_start(
                            out=table[16 * g:16 * g + 16, CPP * s:CPP * (s + 1)],
                            in_=bounce_out[16 * s:16 * s + 16, :])

            if DEBUG:
                nc.sync.dma_start(out=d["dbg_tl"][:], in_=table[0:16, :])
            # ================= prediction =================
            with tc.tile_pool(name="pg", bufs=1) as pgp, \
                 tc.tile_pool(name="posb", bufs=1) as osp, \
                 tc.tile_pool(name="ph", bufs=2) as php, \
                 tc.tile_pool(name="pps", bufs=2, space="PSUM") as ppp, \
                 tc.tile_pool(name="gps", bufs=1, space="PSUM") as gpp:
                ui_a, ui_b = uidx[:, :SLBP // 16], uidx[:, SLBP // 16:]
                NPRED_G = 0 if SKIP_PRED else 1
                pg = pgp.tile([128, SLBP], F32, tag="pg")
                pgb = pgp.tile([128, SLBP], F32, tag="pgb")
                pg2 = pgp.tile([128, SLBP], F32, tag="pg2")
                pg2b = pgp.tile([128, SLBP], F32, tag="pg2b")
                pg3 = pgp.tile([128, SLBP], F32, tag="pg3")
                pg3b = pgp.tile([128, SLBP], F32, tag="pg3b")
                for _ in range(NPRED_G * REP_PRED):
                    nc.gpsimd.ap_gather(pg[:], table[:], ui_a,
                                        channels=128, num_elems=NPOS, d=1, num_idxs=SLBP)
                    nc.gpsimd.ap_gather(pgb[:], table[:], ui_b,
                                        channels=128, num_elems=NPOS, d=1, num_idxs=SLBP)
                    # stds round: overwrite table row 16g with stds, gather
                    for g in range(NG):
                        nc.sync.dma_start(out=table[16 * g:16 * g + 1, :],
                                          in_=d["sm"][0:1, :])
                    nc.gpsimd.ap_gather(pg2[:], table[:], ui_a,
                                        channels=128, num_elems=NPOS, d=1, num_idxs=SLBP)
                    nc.gpsimd.ap_gather(pg2b[:], table[:], ui_b,
                                        channels=128, num_elems=NPOS, d=1, num_idxs=SLBP)
                    # means round
                    for g in range(NG):
                        nc.sync.dma_start(out=table[16 * g:16 * g + 1, :],
                                          in_=d["sm"][1:2, :])
                    nc.gpsimd.ap_gather(pg3[:], table[:], ui_a,
                                        channels=128, num_elems=NPOS, d=1, num_idxs=SLBP)
                    nc.gpsimd.ap_gather(pg3b[:], table[:], ui_b,
                                        channels=128, num_elems=NPOS, d=1, num_idxs=SLBP)

                osbA = osp.tile([128, SLBP], F32, tag="osbA")
                osbB = osp.tile([1, SLBP], F32, tag="osbB")
                NCH = [512] * 6 + [64]
                for j in ([] if SKIP_PRED else list(range(NSL)) * REP_PRED):
                    if j < 3:
                        base, s_pg, s_std, s_mean = 32 * j, pg, pg2, pg3
                        orow = osbA[base:base + 1, :]
                    else:
                        base, s_pg, s_std, s_mean = 0, pgb, pg2b, pg3b
                        orow = osbB[0:1, :]
                    n0 = 0
                    for nn in NCH:
                        hps2 = ppp.tile([HID, 512], F32, tag="hps2")
                        nc.tensor.matmul(hps2[:, :nn], small["fw1"][base:base + 32, :],
                                         s_pg[base:base + 32, n0:n0 + nn])
                        h2 = php.tile([HID, 512], F32, tag="h2")
                        nc.scalar.activation(h2[:, :nn], hps2[:, :nn], RELU,
                                             bias=small["fb1"][:])
                        gps = gpp.tile([128, 512], F32, tag="gps")
                        nc.tensor.matmul(gps[base:base + 1, :nn], small["fw2"][:],
                                         h2[:, :nn])
                        gsl = orow[:, n0:n0 + nn]
                        nc.scalar.activation(gsl, gps[base:base + 1, :nn], RELU,
                                             bias=float(meta["fb2"]))
                        nc.vector.tensor_tensor(gsl, gsl,
                                                s_std[base:base + 1, n0:n0 + nn], MULT)
                        nc.any.tensor_tensor(gsl, gsl,
                                             s_mean[base:base + 1, n0:n0 + nn], ADD)
                        n0 += nn
                    nc.sync.dma_start(out=out_dram[SLBP * j:SLBP * (j + 1)], in_=orow)

    nc.compile()
    return nc


def kernel(**inputs) -> np.ndarray:
    meta, in_maps = _host_prep(inputs)
    nc = _build_program(meta)

    if os.environ.get("BASS_SIM"):
        import concourse.bass_interp as bass_interp
        sim = bass_interp.MultiCoreSim(nc, NCORES)
        for i in range(NCORES):
            for k, v in in_maps[i].items():
                sim.cores[i].tensor(k)[:] = v
        sim.simulate(check_with_hw=False)
        results = [{"out": np.array(sim.cores[i].tensor("out"))} for i in range(NCORES)]
    else:
        res = run_bass_kernel_spmd(nc, in_maps, list(range(NCORES)))
        results = res.results

    outs = []
    for s in range(NCORES):
        o = np.asarray(results[s]["out"], np.float32).reshape(NSL, SLBP)
        outs.append(o[:, :SLB].ravel())
    return np.concatenate(outs)
